# revision 40
# baseline (speedup 1.0000x reference)
"""Bass/Trainium2 kernel for nn_Attn_70076686401576 (block-causal-biased MHA).

Math (per reference):
  qkv = x @ Wqkv + bqkv  -> split into q,k,v heads (H=16, hd=64)
  q,k RMS-normalized over head dim (QKNorm, eps=1e-6, scales gq/gk)
  scores = q k^T / sqrt(hd) + M, where M[i,j] = 1.0 for future-frame keys
  attn = softmax(scores); o = attn @ v; out = o @ Wout + bout

Sharding: 16 heads / 8 cores = 2 heads per core (head-parallel).  Each core
computes its 2 heads' q/k/v from the full x (Wqkv column-sharded), runs full
attention for those heads, and produces a partial output via the row-sharded
Wout.  Host sums the 8 partials (+ bout).

v3 design notes:
  - x loaded ONE DMA per 512-token range ([128,4,1024] f32), cast to bf16 on
    Scalar, transposed by ONE DMA-crossbar transpose per range (the xbar
    transpose blocks its issue queue ~1.2us regardless of size, so batch big)
  - v transposed per-range the same way (2 calls); va tiles strided 80 (the
    xbar needs 16-element-aligned destination offsets)
  - QKNorm chains split across Vector (biases/recip/muls) and Scalar
    (square/sqrt); GpSimd only does memsets (its tensor ops are ~10x slower
    than DVE and it cannot touch PSUM)
  - attention: per-ktile score tile [128, 2head, 512] in PSUM -> ONE scalar
    exp per ktile; scalar exp back-to-back is the phase bound (~285us)
  - softmax denominator: ones-column in V -> po row 64; row copied to SBUF,
    transposed to a [128 tok, tt, h] column layout by a tiny SBUF->SBUF DMA,
    reciprocal'd on Vector, and applied as a per-PARTITION scalar during the
    output-projection PSUM drain (heads kept in separate PSUM tiles) -- no
    PE broadcast matmul, near-zero exp-pipeline stall at stripe boundaries
  - PSUM budget exactly 8 banks: scores 2x2 + po 2 + outproj 2
"""

import math
import ml_dtypes
import numpy as np

N_TOK_FULL = 4096
D_MODEL = 1024
HD = 64
TPF = 256
EPS = 1e-6
N_CORES = 8


def build_program(n_tok=N_TOK_FULL, debug=False):
    import concourse.bass as bass
    import concourse.tile as tile
    from concourse import bacc, mybir
    from contextlib import ExitStack

    f32 = mybir.dt.float32
    f32r = mybir.dt.float32r
    bf16 = mybir.dt.bfloat16
    AF = mybir.ActivationFunctionType
    E_CONST = float(np.exp(1.0))

    D = D_MODEL
    n_ranges = n_tok // 512
    n_ktiles = n_tok // 128
    n_stripes = n_tok // 512

    nc = bacc.Bacc("TRN2", target_bir_lowering=False, debug=False,
                   num_devices=N_CORES)
    x_d = nc.dram_tensor("x", [n_tok, D], bf16, kind="ExternalInput").ap()
    wqkv_d = nc.dram_tensor("wqkv", [D, 384], bf16, kind="ExternalInput").ap()
    bqkv_d = nc.dram_tensor("bqkv", [384], f32, kind="ExternalInput").ap()
    gv_d = nc.dram_tensor("gv", [128, 2], f32, kind="ExternalInput").ap()
    wout_d = nc.dram_tensor("wout", [128, D], bf16, kind="ExternalInput").ap()
    out_d = nc.dram_tensor("out", [n_tok, D], f32, kind="ExternalOutput").ap()
    # DRAM scratch for the denominator transpose (SBUF->SBUF DMAs cannot
    # map a free dim onto partitions; DRAM round-trip can)
    zscr_d = nc.dram_tensor("zscr", [n_tok // 512, 2, 512], f32,
                            kind="Internal").ap()

    out_t = out_d.rearrange("(t p) d -> t p d", p=128)

    dbg = {}
    if debug:
        for nm, shp in (("dbg_oTn0", [64, n_tok]), ("dbg_oTn1", [64, n_tok]),
                        ("dbg_zr0", [128, 8])):
            dbg[nm] = nc.dram_tensor(nm, shp, mybir.dt.float32,
                                     kind="ExternalOutput").ap()

    with tile.TileContext(nc) as tc:
        ctx = ExitStack()
        sb = ctx.enter_context(tc.tile_pool(name="sb", bufs=1))
        sbp = ctx.enter_context(tc.tile_pool(name="sbp", bufs=1))
        sba = ctx.enter_context(tc.tile_pool(name="sba", bufs=1))
        ps1_ctx = ExitStack()
        ps1 = ps1_ctx.enter_context(
            tc.tile_pool(name="ps1", bufs=1, space="PSUM"))
        if True:
            # ---- weights/constants (x is transposed straight from DRAM
            # by the xbar, already bf16 from the host) ----
            wqkv_sb = sb.tile([128, 8, 384], bf16, tag="wqkv")
            nc.gpsimd.dma_start(wqkv_sb,
                                wqkv_d.rearrange("(c p) n -> p c n", p=128))
            bq_sb = sb.tile([128, 3], f32, tag="bq")
            nc.sync.dma_start(bq_sb, bqkv_d.rearrange("(c p) -> p c", p=128))
            gv_sb = sb.tile([128, 2], f32, tag="gv")
            nc.sync.dma_start(gv_sb, gv_d)
            wo0 = sb.tile([64, D], bf16, tag="wo0")
            nc.sync.dma_start(wo0, wout_d[0:64, :])
            wo1 = sb.tile([64, D], bf16, tag="wo1")
            nc.sync.dma_start(wo1, wout_d[64:128, :])

            blkdf = sb.tile([128, 128], f32, tag="blkdf")
            nc.gpsimd.memset(blkdf, 0.0)
            nc.gpsimd.memset(blkdf[0:64, 0:64], 1.0)
            nc.gpsimd.memset(blkdf[64:128, 64:128], 1.0)
            blkdiag = sb.tile([128, 128], f32r, tag="blkdiag")
            nc.vector.tensor_copy(blkdiag, blkdf)
            cb_q = sb.tile([128, 1], f32, tag="cb_q")
            nc.gpsimd.memset(cb_q, 64.0 * EPS)
            cb_k = sb.tile([128, 1], f32, tag="cb_k")
            nc.gpsimd.memset(cb_k, EPS)
            cs_k = sb.tile([128, 1], f32, tag="cs_k")
            nc.gpsimd.memset(cs_k, 1.0 / 64.0)

            # ---- persistent attention operands ----
            qTb = sb.tile([128, n_tok], bf16, tag="qTb")
            kTb = sb.tile([128, n_tok], bf16, tag="kTb")
            vTa = sb.tile([128, n_tok], bf16, tag="vTa")
            # stride 80 (not 65): xbar-transpose writes need 16-element
            # aligned destination offsets
            va0 = sb.tile([128, n_ktiles, 80], bf16, tag="va0")
            va1 = sb.tile([128, n_ktiles, 80], bf16, tag="va1")
            eva0 = sb.tile([128, n_ktiles, 80], bf16, tag="eva0")
            eva1 = sb.tile([128, n_ktiles, 80], bf16, tag="eva1")
            nc.gpsimd.memset(va0[:, :, 64:65], 1.0)
            nc.gpsimd.memset(va1[:, :, 64:65], 1.0)
            nc.gpsimd.memset(eva0[:, :, 64:65], E_CONST)
            nc.gpsimd.memset(eva1[:, :, 64:65], E_CONST)

            # ========== phase 1: projection + QKNorm + STRIPE-0 attention ====
            # One 8-bank PSUM pool for everything: scores sg 2x2 + po 2 +
            # "pso" 2 (outproj later; during phase 1 the projection and the
            # RMS matmuls pass through the pso rotation, which is free until
            # stripe 1's output projection).  Stripe 0's attention interleaves
            # with the projection: ktiles of range r run right after range
            # r's QKNorm, so the first 32 exps hide under phase 1.

            oTn0 = sba.tile([64, n_tok], bf16, tag="oTn0")
            oTn1 = sba.tile([64, n_tok], bf16, tag="oTn1")

            def emit_proj(r):
                # ONE xbar transpose per range, straight from DRAM:
                # x[512 tok, 1024] -> xTr [128, dc, 512 tok]
                xTr = sbp.tile([128, 8, 512], bf16, tag="xT", bufs=3,
                               name=f"xTr_{r}")
                nc.sync.dma_start_transpose(
                    xTr, x_d[r * 512:(r + 1) * 512, :])
                sl = slice(r * 512, (r + 1) * 512)
                qkr = []
                for oc in range(3):
                    pjt = ps1.tile([128, 512], f32, tag=f"pj{oc}", bufs=2,
                                   name=f"pj_{r}_{oc}")
                    for dc in range(8):
                        nc.tensor.matmul(
                            pjt,
                            wqkv_sb[:, dc, oc * 128:(oc + 1) * 128],
                            xTr[:, dc, :],
                            start=(dc == 0), stop=(dc == 7))
                    if oc == 2:
                        nc.vector.tensor_scalar_add(vTa[:, sl], pjt,
                                                    bq_sb[:, 2:3])
                    else:
                        t = sbp.tile([128, 512], f32r,
                                     tag=("qTr", "kTr")[oc], bufs=2,
                                     name=f"{('q', 'k')[oc]}Tr_{r}")
                        nc.vector.tensor_scalar_add(t, pjt,
                                                    bq_sb[:, oc:oc + 1])
                        qkr.append(t)
                return qkr

            def emit_qknorm_va(r, qTr, kTr):
                sl = slice(r * 512, (r + 1) * 512)
                for which, blk, blkb in (("q", qTr, qTb), ("k", kTr, kTb)):
                    sq = sbp.tile([128, 512], f32r, tag=f"sq{which}", bufs=2,
                                  name=f"sq_{r}_{which}")
                    nc.scalar.activation(sq, blk, AF.Square)
                    ps_r = ps1.tile([128, 512], f32, tag="psr", bufs=2,
                                    name=f"psr_{r}_{which}")
                    nc.tensor.matmul(ps_r, blkdiag, sq, start=True, stop=True)
                    sqs = sbp.tile([128, 512], f32, tag=f"sqs{which}", bufs=2,
                                   name=f"sqs_{r}_{which}")
                    if which == "q":
                        nc.scalar.activation(sqs, ps_r, AF.Sqrt,
                                             bias=cb_q, scale=1.0)
                    else:
                        nc.scalar.activation(sqs, ps_r, AF.Sqrt,
                                             bias=cb_k, scale=cs_k)
                    rs = sbp.tile([128, 512], f32, tag=f"rs{which}", bufs=2,
                                  name=f"rs_{r}_{which}")
                    nc.vector.reciprocal_approx_fast(rs, sqs)
                    gcol = 0 if which == "q" else 1
                    nc.vector.tensor_scalar_mul(rs, rs,
                                                gv_sb[:, gcol:gcol + 1])
                    nc.vector.tensor_mul(blkb[:, sl], blk, rs)

            vab = (va0, va1)
            vab = (va0, va1)
            evab = (eva0, eva1)

            def emit_norm(s, po):
                """Free po: copy unnormalized o to SBUF + extract denoms.

                The denominator row (64) of each head's po is copied into
                zrow ([65, 2, 512]: head on the middle dim), then a tiny
                SBUF->SBUF DMA transposes both rows into zcol
                [128 tok, tt, h] and Vector reciprocals it.  The division
                happens later, during the outproj PSUM drain, as a
                per-partition (=per-token) scalar."""
                qsl = slice(s * 512, (s + 1) * 512)
                nc.vector.tensor_copy(oTn0[:, qsl], po[0][0:64, :])
                nc.vector.tensor_copy(oTn1[:, qsl], po[1][0:64, :])
                zrow = sba.tile([65, 2, 512], f32, tag="zrow", bufs=2,
                                name=f"zrow_{s}")
                nc.vector.tensor_copy(zrow[64:65, 0, :], po[0][64:65, :])
                nc.vector.tensor_copy(zrow[64:65, 1, :], po[1][64:65, :])
                # transpose [2, 512] -> [128 tok, 2, 4] via DRAM round-trip
                nc.sync.dma_start(zscr_d[s], zrow[64:65, :, :])
                zcol = sba.tile([128, 2, 4], f32, tag="zcol", bufs=2,
                                name=f"zcol_{s}")
                nc.sync.dma_start(
                    zcol,
                    zscr_d[s].rearrange("h (t p) -> p h t", p=128))
                zr = sba.tile([128, 2, 4], f32, tag="zr", bufs=2,
                              name=f"zr_{s}")
                nc.vector.reciprocal_approx_fast(zr, zcol)
                if debug and s == 0:
                    nc.sync.dma_start(
                        dbg["dbg_zr0"], zr.rearrange("p a b -> p (a b)"))
                return zr

            op_state = {}

            def emit_outproj_mm(s, zr, tt, half, which, tag="pso"):
                """One head's outproj matmul for (token-tile, dmodel-half);
                which=1 also drains: the two heads' PSUMs are combined with
                the per-token 1/Z scalars."""
                t0 = s * 512 + tt * 128
                gt = s * 4 + tt
                nsl = slice(half * 512, (half + 1) * 512)
                nb = 2 if tag == "pso" else 1
                if which == 0:
                    ps_a = ps2.tile([128, 512], f32, tag=tag, bufs=nb,
                                    name=f"psa_{s}_{tt}_{half}")
                    nc.tensor.matmul(ps_a, oTn0[:, t0:t0 + 128], wo0[:, nsl],
                                     start=True, stop=True)
                    op_state[(s, tt, half)] = ps_a
                    return
                ps_a = op_state.pop((s, tt, half))
                ps_b = ps2.tile([128, 512], f32, tag=tag, bufs=nb,
                                name=f"psb_{s}_{tt}_{half}")
                nc.tensor.matmul(ps_b, oTn1[:, t0:t0 + 128], wo1[:, nsl],
                                 start=True, stop=True)
                tmp = sba.tile([128, 512], f32, tag="obt", bufs=2,
                               name=f"obt_{s}_{tt}_{half}")
                if tag == "pso":
                    nc.vector.tensor_scalar_mul(tmp, ps_b,
                                                zr[:, 1, tt:tt + 1])
                else:
                    # tail: scalar engine is idle after the last exp
                    nc.scalar.activation(tmp, ps_b, AF.Copy,
                                         scale=zr[:, 1, tt:tt + 1])
                ob = sba.tile([128, 512], f32, tag="ob", bufs=4,
                              name=f"ob_{s}_{tt}_{half}")
                nc.vector.scalar_tensor_tensor(
                    ob, ps_a, zr[:, 0, tt:tt + 1], tmp,
                    op0=mybir.AluOpType.mult, op1=mybir.AluOpType.add)
                nc.sync.dma_start(out_t[gt][:, nsl], ob)

            def attn_ktile(s, po, kt):
                qsl = slice(s * 512, (s + 1) * 512)
                sg = ps2.tile([128, 2, 512], f32, tag="sg", bufs=2,
                              name=f"sg_{s}_{kt}")
                for h in range(2):
                    hp = slice(h * 64, (h + 1) * 64)
                    nc.tensor.matmul(
                        sg[:, h, :],
                        kTb[hp, kt * 128:(kt + 1) * 128],
                        qTb[hp, qsl],
                        start=True, stop=True,
                        tile_position=(h * 64, 0))
                et = sba.tile([128, 2, 512], bf16, tag="et", bufs=6,
                              name=f"et_{s}_{kt}")
                nc.scalar.activation(et, sg, AF.Exp)

                fk = kt // 2
                first = (kt == 0)
                last = (kt == n_ktiles - 1)
                for h in range(2):
                    rhs = et[:, h, :]
                    if fk == 2 * s + 1:
                        # key frame == 2nd query frame of the stripe:
                        # first 256 queries see it as future (e*V)
                        nc.tensor.matmul(
                            po[h][:, 0:256],
                            evab[h][:, kt, 0:65],
                            rhs[:, 0:256],
                            start=False, stop=False)
                        # stop only on the final matmul (the whole
                        # [65,512] tile is one 2KB psum zero region)
                        nc.tensor.matmul(
                            po[h][:, 256:512],
                            vab[h][:, kt, 0:65],
                            rhs[:, 256:512],
                            start=False, stop=last)
                    else:
                        vv = evab[h] if fk > 2 * s + 1 else vab[h]
                        nc.tensor.matmul(
                            po[h][:, :],
                            vv[:, kt, 0:65],
                            rhs,
                            start=first, stop=last)

            # --- phase 1, software-pipelined by one range ---
            prev = None
            for r in range(n_ranges):
                qkr = emit_proj(r)
                if prev is not None:
                    emit_qknorm_va(*prev)
                prev = (r, qkr[0], qkr[1])
                if r == n_ranges // 2 - 1:
                    # first-half va batch (vTa ranges 0..3 are written)
                    h2 = n_tok // 2
                    nc.sync.dma_start_transpose(
                        va0[:, 0:n_ktiles // 2, 0:64], vTa[0:64, 0:h2])
                    nc.sync.dma_start_transpose(
                        va1[:, 0:n_ktiles // 2, 0:64], vTa[64:128, 0:h2])
                    nc.vector.tensor_scalar_mul(
                        eva0[:, 0:n_ktiles // 2, 0:64],
                        va0[:, 0:n_ktiles // 2, 0:64], E_CONST)
                    nc.vector.tensor_scalar_mul(
                        eva1[:, 0:n_ktiles // 2, 0:64],
                        va1[:, 0:n_ktiles // 2, 0:64], E_CONST)

            # V -> va/eva: second-half batch (the first half was issued
            # mid-loop); e-scales per half
            h2 = n_tok // 2
            nc.sync.dma_start_transpose(va0[:, n_ktiles // 2:, 0:64],
                                        vTa[0:64, h2:])
            nc.sync.dma_start_transpose(va1[:, n_ktiles // 2:, 0:64],
                                        vTa[64:128, h2:])
            nc.vector.tensor_scalar_mul(eva0[:, n_ktiles // 2:, 0:64],
                                        va0[:, n_ktiles // 2:, 0:64],
                                        E_CONST)
            nc.vector.tensor_scalar_mul(eva1[:, n_ktiles // 2:, 0:64],
                                        va1[:, n_ktiles // 2:, 0:64],
                                        E_CONST)

            emit_qknorm_va(*prev)

            ps1_ctx.close()
            ps2_ctx = ExitStack()
            ps2 = ps2_ctx.enter_context(
                tc.tile_pool(name="ps2", bufs=1, space="PSUM"))

            # --- attention: exp-paced steady state ---
            pending = None
            for s in range(n_stripes):
                po = [ps2.tile([128, 512], f32, tag=f"po{h}", bufs=1,
                               name=f"po{h}_{s}")[0:65, :]
                      for h in range(2)]
                for kt in range(n_ktiles):
                    attn_ktile(s, po, kt)
                    # previous stripe's outproj, spread ONE matmul per
                    # ktile so the exp pipeline never starves
                    if pending is not None and 4 <= kt < 20:
                        j = kt - 4
                        emit_outproj_mm(pending[0], pending[1],
                                        (j // 2) // 2, (j // 2) % 2, j % 2)
                        if kt == 19:
                            pending = None
                # free po quickly: unnormalized o + denominators out
                zr_s = emit_norm(s, po)
                pending = (s, zr_s)
            tags = ("pso", "po0", "po1")
            for i, (tt, half) in enumerate(
                    (t, h) for t in range(4) for h in range(2)):
                emit_outproj_mm(pending[0], pending[1], tt, half, 0,
                                tag=tags[(2 * i) % 3])
                emit_outproj_mm(pending[0], pending[1], tt, half, 1,
                                tag=tags[(2 * i + 1) % 3])

            if debug:
                for nm, t in (("dbg_oTn0", oTn0), ("dbg_oTn1", oTn1)):
                    stg = sba.tile([64, n_tok], f32, tag=f"stg_{nm}")
                    nc.scalar.copy(stg, t)
                    nc.sync.dma_start(dbg[nm], stg)

            ps2_ctx.close()
            ctx.close()

    nc.compile()
    return nc


def shard_inputs(x, Wqkv, bqkv, gq, gk, Wout, n_tok):
    """Build the 8 per-core input maps (head-parallel sharding)."""
    D = D_MODEL
    in_maps = []
    for c in range(N_CORES):
        cs = slice(128 * c, 128 * (c + 1))
        wq = Wqkv[:, cs]
        wk = Wqkv[:, D + 128 * c:D + 128 * (c + 1)]
        wv = Wqkv[:, 2 * D + 128 * c:2 * D + 128 * (c + 1)]
        wqkv_s = np.ascontiguousarray(np.concatenate([wq, wk, wv], axis=1),
                                      dtype=np.float32)
        bq = bqkv[cs]
        bk = bqkv[D + 128 * c:D + 128 * (c + 1)]
        bv = bqkv[2 * D + 128 * c:2 * D + 128 * (c + 1)]
        bqkv_s = np.ascontiguousarray(np.concatenate([bq, bk, bv]),
                                      dtype=np.float32)
        gv = np.stack([np.concatenate([gq, gq]),
                       np.concatenate([gk, gk])], axis=1).astype(np.float32)
        wout_s = np.ascontiguousarray(Wout[cs, :], dtype=np.float32)
        in_maps.append({
            "x": np.ascontiguousarray(
                x[:n_tok].astype(ml_dtypes.bfloat16)),
            "wqkv": wqkv_s.astype(ml_dtypes.bfloat16),
            "bqkv": bqkv_s,
            "gv": np.ascontiguousarray(gv),
            "wout": wout_s.astype(ml_dtypes.bfloat16),
        })
    return in_maps


_PROGRAM_CACHE = {}


def _get_program(n_tok):
    if n_tok not in _PROGRAM_CACHE:
        _PROGRAM_CACHE[n_tok] = build_program(n_tok)
    return _PROGRAM_CACHE[n_tok]


def run_sharded(inputs, trace=False, tmpdir=None):
    """Run the SPMD kernel; returns (full_output [1,N,D], BassKernelResults)."""
    from concourse.bass_utils import run_bass_kernel_spmd

    x = np.asarray(inputs["x"], dtype=np.float32)
    Wqkv = np.asarray(inputs["Wqkv"], dtype=np.float32)
    bqkv = np.asarray(inputs["bqkv"], dtype=np.float32)
    Wout = np.asarray(inputs["Wout"], dtype=np.float32)
    bout = np.asarray(inputs["bout"], dtype=np.float32)
    gq = np.asarray(inputs["gq"], dtype=np.float32)
    gk = np.asarray(inputs["gk"], dtype=np.float32)
    tpf = int(np.asarray(inputs["tokens_per_frame"]))
    assert tpf == TPF, f"kernel hardcodes tokens_per_frame={TPF}, got {tpf}"

    B, N, D = x.shape
    assert B == 1 and D == D_MODEL
    x2 = x[0]

    nc = _get_program(N)
    in_maps = shard_inputs(x2, Wqkv, bqkv, gq, gk, Wout, N)
    res = run_bass_kernel_spmd(nc, in_maps, list(range(N_CORES)),
                               trace=trace, tmpdir=tmpdir)
    acc = res.results[0]["out"].astype(np.float32)
    for c in range(1, N_CORES):
        acc = acc + res.results[c]["out"]
    if np.any(bout):
        acc = acc + bout[None, :]
    return acc[None], res


def kernel(**inputs):
    out, _ = run_sharded(inputs)
    return out
